# revision 1
# baseline (speedup 1.0000x reference)
"""GraphWeightedMHA on 8 trn2 cores — sequence-sharded Bass/Tile kernel.

Sharding: rows of N=4096 split 512/core. Each core projects q/k/v for its rows,
AllGathers k^T and v, computes softmax(q k^T/sqrt(d)) v for its 512 query rows
(transposed layout: S^T tiles via PE, exp on ACT, PV accumulated in PSUM with a
ones-augmented V to get softmax denominators), AllGathers the normalized
attention output, then computes its row-block of sgconv_mat @ attn and the
final output projection. All big matmuls run in float32r (~2^-12 precision,
full PE rate); accumulation is fp32 in PSUM.
"""
import numpy as np
import concourse.bass as bass
import concourse.bacc as bacc
import concourse.tile as tile
from concourse import mybir
from concourse.bass_utils import run_bass_kernel_spmd

dt = mybir.dt
NC = 8
N, D, H, HD = 4096, 512, 8, 64
RS = N // NC            # 512 rows per core
NB = D // 128           # 4 blocks of 128 along D
KT = N // 128           # 32 key tiles
SCALE = float(1.0 / np.sqrt(np.float32(D)))
Exp = mybir.ActivationFunctionType.Exp
Ident = mybir.ActivationFunctionType.Identity


def round_fp32r(x: np.ndarray) -> np.ndarray:
    u = np.ascontiguousarray(x, dtype=np.float32).view(np.uint32)
    r = (u + np.uint32(0x7FF) + ((u >> np.uint32(12)) & np.uint32(1))) & np.uint32(0xFFFFF000)
    return r.view(np.float32)


def blk(x):  # [D, M] -> [128, NB, M]  (p, kb, m) with d = kb*128+p
    return np.ascontiguousarray(x.reshape(NB, 128, -1).transpose(1, 0, 2))


_CACHE: dict = {}


def _build():
    nc = bacc.Bacc("TRN2", target_bir_lowering=False, debug=False, num_devices=NC)

    def din(name, shape, d=dt.float32r):
        return nc.dram_tensor(name, shape, d, kind="ExternalInput").ap()

    qT_d = din("qT", [128, NB, RS])       # query^T shard, blocked
    kTi_d = din("kTi", [128, NB, RS])     # key^T shard
    vTi_d = din("vTi", [128, NB, RS])     # value^T shard
    wq_d = din("wq", [128, NB, D])        # Wq^T blocked
    wk_d = din("wk", [128, NB, D])
    wv_d = din("wv", [128, NB, D])
    wo_d = din("wo", [128, NB, D])
    bq_d = din("bq", [128, NB], dt.float32)
    bk_d = din("bk", [128, NB], dt.float32)
    bv_d = din("bv", [1, D])
    bo_d = din("bo", [1, D])
    ones_d = din("ones", [1, D])
    onescol_d = din("onescol", [128, KT, 1])
    eye_d = din("eye", [128, 128], dt.float32)
    sgT_d = din("sgT", [N, RS])           # sgconv_mat[rows,:].T per core
    out_d = nc.dram_tensor("out", [RS, D], dt.float32, kind="ExternalOutput").ap()

    with tile.TileContext(nc) as tc:
        with tc.tile_pool(name="const", bufs=1) as cp, \
             tc.tile_pool(name="persist", bufs=1) as pp, \
             tc.tile_pool(name="dram", bufs=1, space="DRAM") as dp:
            wq_sb = cp.tile([128, NB, D], dt.float32r)
            wk_sb = cp.tile([128, NB, D], dt.float32r)
            wv_sb = cp.tile([128, NB, D], dt.float32r)
            wo_sb = cp.tile([128, NB, D], dt.float32r)
            bq_sb = cp.tile([128, NB], dt.float32)
            bk_sb = cp.tile([128, NB], dt.float32)
            bv_sb = cp.tile([1, D], dt.float32r)
            bo_sb = cp.tile([1, D], dt.float32r)
            ones_sb = cp.tile([1, D], dt.float32r)
            eye_sb = cp.tile([128, 128], dt.float32)
            for sb_t, d_t in [(wq_sb, wq_d), (wk_sb, wk_d), (wv_sb, wv_d),
                              (wo_sb, wo_d), (bq_sb, bq_d), (bk_sb, bk_d),
                              (bv_sb, bv_d), (bo_sb, bo_d), (ones_sb, ones_d),
                              (eye_sb, eye_d)]:
                nc.sync.dma_start(sb_t[:], d_t[:])

            qT_sb = pp.tile([128, NB, RS], dt.float32r)    # [p, mb, n] dout=mb*128+p
            attn_sb = pp.tile([128, NB, D], dt.float32r)   # [p, qs, d]  n=qs*128+p

            kb_bounce = dp.tile([RS, D], dt.float32r)
            vb_bounce = dp.tile([RS, D], dt.float32r)
            kb_gath = dp.tile([N, D], dt.float32r, addr_space="Shared")
            vb_gath = dp.tile([N, D], dt.float32r, addr_space="Shared")
            attn_bounce_a = dp.tile([RS, D // 2], dt.float32r)
            attn_bounce_b = dp.tile([RS, D // 2], dt.float32r)
            attn_gath_a = dp.tile([N, D // 2], dt.float32r, addr_space="Shared")
            attn_gath_b = dp.tile([N, D // 2], dt.float32r, addr_space="Shared")
            sgb = pp.tile([128, KT, RS], dt.float32r)
            nc.sync.dma_start(sgb[:], sgT_d[:].rearrange("(jt p) i -> p jt i", jt=KT))

            # ---------------- Phase A: projections ----------------
            with tc.tile_pool(name="pa_sb", bufs=1) as pa_sb, \
                 tc.tile_pool(name="pa_ps", bufs=2, space="PSUM") as pa_ps:
                quT = pa_sb.tile([128, NB, RS], dt.float32r)
                keT = pa_sb.tile([128, NB, RS], dt.float32r)
                vaT = pa_sb.tile([128, NB, RS], dt.float32r)
                nc.sync.dma_start(keT[:], kTi_d[:])
                nc.sync.dma_start(vaT[:], vTi_d[:])
                nc.sync.dma_start(quT[:], qT_d[:])

                kT_sb = pa_sb.tile([128, NB, RS], dt.float32r)
                v_sb = pa_sb.tile([128, NB, D], dt.float32r)

                # k^T = Wk @ key^T + bk  (out [dout, n])
                for mb in range(NB):
                    ps = pa_ps.tile([128, RS], dt.float32, tag="pa")
                    for kb in range(NB):
                        nc.tensor.matmul(ps[:], wk_sb[:, kb, mb * 128:(mb + 1) * 128],
                                         keT[:, kb, :], start=(kb == 0), stop=(kb == NB - 1))
                    nc.scalar.activation(kT_sb[:, mb, :], ps[:], Ident,
                                         bias=bk_sb[:, mb:mb + 1])
                # v = value @ Wv^T + bv  (out [n, dout])
                for nb in range(NB):
                    ps = pa_ps.tile([128, D], dt.float32, tag="pa")
                    for kb in range(NB):
                        nc.tensor.matmul(ps[:], vaT[:, kb, nb * 128:(nb + 1) * 128],
                                         wv_sb[:, kb, :], start=(kb == 0), stop=False)
                    nc.tensor.matmul(ps[:], ones_sb[0:1, 0:128], bv_sb[:],
                                     start=False, stop=True)
                    nc.vector.tensor_copy(v_sb[:, nb, :], ps[:])
                nc.sync.dma_start(
                    kb_bounce[:].rearrange("(mb p) n -> p mb n", mb=NB), kT_sb[:])
                nc.gpsimd.collective_compute(
                    "AllGather", mybir.AluOpType.bypass,
                    replica_groups=[list(range(NC))],
                    ins=[kb_bounce[:].opt()], outs=[kb_gath[:].opt()])
                nc.sync.dma_start(
                    vb_bounce[:].rearrange("(nb p) n -> p nb n", nb=NB), v_sb[:])
                nc.gpsimd.collective_compute(
                    "AllGather", mybir.AluOpType.bypass,
                    replica_groups=[list(range(NC))],
                    ins=[vb_bounce[:].opt()], outs=[vb_gath[:].opt()])
                # q^T = Wq @ query^T + bq
                for mb in range(NB):
                    ps = pa_ps.tile([128, RS], dt.float32, tag="pa")
                    for kb in range(NB):
                        nc.tensor.matmul(ps[:], wq_sb[:, kb, mb * 128:(mb + 1) * 128],
                                         quT[:, kb, :], start=(kb == 0), stop=(kb == NB - 1))
                    nc.scalar.activation(qT_sb[:, mb, :], ps[:], Ident,
                                         bias=bq_sb[:, mb:mb + 1])

            # ---------------- Phase B: attention ----------------
            kv_r = kb_gath[:].rearrange("(i r) n -> r i n", i=NC)        # [512, 8, 512]
            v_r = vb_gath[:].rearrange("(i nb p) n -> p i nb n", i=NC, nb=NB)
            with tc.tile_pool(name="kp", bufs=2) as kp_pool, \
                 tc.tile_pool(name="vh", bufs=2) as vh_pool, \
                 tc.tile_pool(name="pt", bufs=3) as pt_pool, \
                 tc.tile_pool(name="ot", bufs=2) as ot_pool, \
                 tc.tile_pool(name="sc", bufs=4) as sc_pool, \
                 tc.tile_pool(name="s_ps", bufs=2, space="PSUM") as s_ps_pool, \
                 tc.tile_pool(name="o_ps", bufs=1, space="PSUM") as o_ps_pool, \
                 tc.tile_pool(name="t_ps", bufs=1, space="PSUM") as t_ps_pool:
                for pair in range(H // 2):
                    kpair = kp_pool.tile([128, NC, RS], dt.float32r, tag="kp")
                    nc.sync.dma_start(kpair[:], kv_r[pair * 128:(pair + 1) * 128, :, :])
                    for sub in range(2):
                        h = pair * 2 + sub
                        base = sub * 64
                        vh = vh_pool.tile([128, KT, HD + 1], dt.float32r, tag="vh")
                        for i in range(NC):
                            nc.sync.dma_start(
                                vh[:, i * NB:(i + 1) * NB, 0:HD],
                                v_r[:, i, :, h * HD:(h + 1) * HD])
                        nc.sync.dma_start(vh[:, :, HD:HD + 1], onescol_d[:])
                        o_ps = o_ps_pool.tile([HD + 1, RS], dt.float32, tag="ops")
                        qh = qT_sb[base:base + 64, h // 2, :]
                        for g in range(11):
                            sz = 3 if g < 10 else 2
                            s_ps = s_ps_pool.tile([128, 3 * RS], dt.float32, tag="sps")
                            for t in range(sz):
                                kt = g * 3 + t
                                nc.tensor.matmul(
                                    s_ps[:, t * RS:(t + 1) * RS],
                                    kpair[base:base + 64, kt // NB,
                                          (kt % NB) * 128:(kt % NB) * 128 + 128],
                                    qh, start=True, stop=True)
                            p_sb = pt_pool.tile([128, 3 * RS], dt.float32r, tag="pt")
                            nc.scalar.activation(p_sb[:, 0:sz * RS], s_ps[:, 0:sz * RS],
                                                 Exp, scale=SCALE)
                            for t in range(sz):
                                kt = g * 3 + t
                                nc.tensor.matmul(
                                    o_ps[:], vh[:, kt, :], p_sb[:, t * RS:(t + 1) * RS],
                                    start=(kt == 0), stop=(kt == KT - 1),
                                    skip_group_check=True)
                        ot = ot_pool.tile([HD + 1, RS], dt.float32, tag="ot")
                        nc.vector.tensor_copy(ot[:], o_ps[:])
                        for qs in range(NB):
                            t_ps = t_ps_pool.tile([128, HD + 1], dt.float32, tag="tps")
                            nc.tensor.transpose(t_ps[:], ot[:, qs * 128:(qs + 1) * 128],
                                                eye_sb[0:HD + 1, 0:HD + 1])
                            rc = sc_pool.tile([128, 1], dt.float32, tag="rc")
                            nc.vector.reciprocal(rc[:], t_ps[:, HD:HD + 1])
                            nc.vector.tensor_scalar_mul(
                                attn_sb[:, qs, h * HD:(h + 1) * HD], t_ps[:, 0:HD], rc[:])
                    if pair == 1:
                        nc.sync.dma_start(
                            attn_bounce_a[:].rearrange("(qs p) d -> p qs d", qs=NB),
                            attn_sb[:, :, 0:D // 2])
                        nc.gpsimd.collective_compute(
                            "AllGather", mybir.AluOpType.bypass,
                            replica_groups=[list(range(NC))],
                            ins=[attn_bounce_a[:].opt()], outs=[attn_gath_a[:].opt()])
                    if pair == 3:
                        nc.sync.dma_start(
                            attn_bounce_b[:].rearrange("(qs p) d -> p qs d", qs=NB),
                            attn_sb[:, :, D // 2:D])
                        nc.gpsimd.collective_compute(
                            "AllGather", mybir.AluOpType.bypass,
                            replica_groups=[list(range(NC))],
                            ins=[attn_bounce_b[:].opt()], outs=[attn_gath_b[:].opt()])

            # ---------------- Phase C: sgconv (out_sg^T) ----------------
            with tc.tile_pool(name="aj", bufs=3) as aj_pool, \
                 tc.tile_pool(name="og_ps", bufs=1, space="PSUM") as og_pool, \
                 tc.tile_pool(name="pd_sb", bufs=1) as pd_sb_pool:
                og = og_pool.tile([128, NB, RS], dt.float32)   # 4 banks: [d_sub, n]
                for half, gath in ((0, attn_gath_a), (1, attn_gath_b)):
                    for jt in range(KT):
                        aj = aj_pool.tile([128, D // 2], dt.float32r, tag="aj")
                        nc.sync.dma_start(aj[:], gath[jt * 128:(jt + 1) * 128, :])
                        for dbl in range(2):
                            db = half * 2 + dbl
                            nc.tensor.matmul(og[:, db, :],
                                             aj[:, dbl * 128:(dbl + 1) * 128],
                                             sgb[:, jt, :],
                                             start=(jt == 0), stop=(jt == KT - 1),
                                             skip_group_check=True)
                # ---------------- Phase D: final projection ----------------
                ogT = pd_sb_pool.tile([128, NB, RS], dt.float32r)
                for db in range(NB):
                    nc.vector.tensor_copy(ogT[:, db, :], og[:, db, :])
                with tc.tile_pool(name="pd_ps", bufs=2, space="PSUM") as pd_ps_pool, \
                     tc.tile_pool(name="po_sb", bufs=2) as po_sb_pool:
                    for nb in range(NB):
                        ps = pd_ps_pool.tile([128, D], dt.float32, tag="pd")
                        for db in range(NB):
                            nc.tensor.matmul(ps[:], ogT[:, db, nb * 128:(nb + 1) * 128],
                                             wo_sb[:, db, :], start=(db == 0), stop=False)
                        nc.tensor.matmul(ps[:], ones_sb[0:1, 0:128], bo_sb[:],
                                         start=False, stop=True)
                        po = po_sb_pool.tile([128, D], dt.float32, tag="po")
                        nc.vector.tensor_copy(po[:], ps[:])
                        nc.sync.dma_start(out_d[nb * 128:(nb + 1) * 128, :], po[:])
    nc.compile()
    return nc


def kernel(**inputs):
    query = np.asarray(inputs["query"], dtype=np.float32)
    key = np.asarray(inputs["key"], dtype=np.float32)
    value = np.asarray(inputs["value"], dtype=np.float32)
    Wq, bq = np.asarray(inputs["Wq"], np.float32), np.asarray(inputs["bq"], np.float32)
    Wk, bk = np.asarray(inputs["Wk"], np.float32), np.asarray(inputs["bk"], np.float32)
    Wv, bv = np.asarray(inputs["Wv"], np.float32), np.asarray(inputs["bv"], np.float32)
    Wo, bo = np.asarray(inputs["Wo"], np.float32), np.asarray(inputs["bo"], np.float32)
    sg = np.asarray(inputs["sgconv_mat"], np.float32)[0]   # [N, N]

    if "nc" not in _CACHE:
        _CACHE["nc"] = _build()
    nc = _CACHE["nc"]

    qT = round_fp32r(query[0].T)   # [D, N]
    kT = round_fp32r(key[0].T)
    vT = round_fp32r(value[0].T)
    wq_b = blk(round_fp32r(Wq.T))
    wk_b = blk(round_fp32r(Wk.T))
    wv_b = blk(round_fp32r(Wv.T))
    wo_b = blk(round_fp32r(Wo.T))
    bq_b = np.ascontiguousarray(bq.reshape(NB, 128).T)
    bk_b = np.ascontiguousarray(bk.reshape(NB, 128).T)
    common = {
        "wq": wq_b, "wk": wk_b, "wv": wv_b, "wo": wo_b,
        "bq": bq_b, "bk": bk_b,
        "bv": round_fp32r(bv.reshape(1, D)), "bo": round_fp32r(bo.reshape(1, D)),
        "ones": np.ones((1, D), np.float32),
        "onescol": np.ones((128, KT, 1), np.float32),
        "eye": np.eye(128, dtype=np.float32),
    }
    in_maps = []
    for c in range(NC):
        sl = slice(c * RS, (c + 1) * RS)
        in_maps.append(dict(
            common,
            qT=blk(qT[:, sl]), kTi=blk(kT[:, sl]), vTi=blk(vT[:, sl]),
            sgT=round_fp32r(sg[sl, :].T),
        ))
    res = run_bass_kernel_spmd(nc, in_maps, core_ids=list(range(NC)),
                               **_CACHE.get("run_kwargs", {}))
    _CACHE["last_results"] = res
    out = np.concatenate([res.results[c]["out"] for c in range(NC)], axis=0)
    return out.reshape(1, N, D)



# revision 2
# speedup vs baseline: 1.8283x; 1.8283x over previous
"""GraphWeightedMHA on 8 trn2 cores — head-sharded bf16 Bass/Tile kernel.

Sharding: one attention head per core (tensor parallel). Each core projects
q/k/v for ALL 4096 sequence positions but only its head's 64 dims — so no
k/v collective is needed at all. Attention (S^T tiles via PE with 2-way
row-packing, exp split between ACT and a DVE fast-exp, PV with a
ones-augmented V for softmax denominators) produces this head's [4096, 64]
output slice, which is AllGathered (in two halves, overlapped with compute)
to form the full [4096, 512] attention matrix. The sgconv matmul and final
projection are then row-sharded: core c computes output rows [c*512,(c+1)*512).

All matmuls are bf16 (fp32 PSUM accumulation); fp32r would compile to the
~3x slower fp32_mode=HIGH multi-pass path.
"""
import numpy as np
import ml_dtypes
import concourse.bass as bass
import concourse.bacc as bacc
import concourse.tile as tile
from concourse import mybir
from concourse.bass_utils import run_bass_kernel_spmd

dt = mybir.dt
bf16 = ml_dtypes.bfloat16
NC = 8
N, D, H, HD = 4096, 512, 8, 64
RS = N // NC          # 512 output rows per core for sgconv/final proj
NB = D // 128         # 4 blocks of 128 along D
QB = 8                # query blocks of 512
KT = N // 128         # 32 key tiles
NPAIR = KT // 2       # 16 row-packed S^T pairs per query block
SCALE = float(1.0 / np.sqrt(np.float32(D)))
Exp = mybir.ActivationFunctionType.Exp
Ident = mybir.ActivationFunctionType.Identity
Mult = mybir.AluOpType.mult
Add = mybir.AluOpType.add

# Schraudolph fast-exp in bf16 bit space: bf16_bits(exp(x)) ~ round(A*x + B)
FEXP_A = float(np.float32(128.0 / np.log(2.0)))
FEXP_B = float(np.float32(127 * 128 - 5.0))
# pairs g < ACT_PAIRS use the scalar engine's exact exp; the rest use the
# DVE fast-exp. Balances ACT vs DVE busy time.
ACT_PAIRS = 6

_CACHE: dict = {}


def blk(x):  # [512, M] -> [128, 4, M]  (p, kb, m) with d = kb*128+p
    return np.ascontiguousarray(x.reshape(NB, 128, -1).transpose(1, 0, 2))


def _build():
    nc = bacc.Bacc("TRN2", target_bir_lowering=False, debug=False, num_devices=NC)

    def din(name, shape, d=dt.bfloat16):
        return nc.dram_tensor(name, shape, d, kind="ExternalInput").ap()

    qT_d = din("qT", [128, NB, N])        # query^T blocked (shared)
    kT_d = din("kT", [128, NB, N])        # key^T blocked (shared)
    vT_d = din("vT", [128, NB, N])        # value^T blocked (shared)
    wq_d = din("wq", [128, NB, HD])       # (s*Wq_h)^T blocked
    wk_d = din("wk", [128, NB, HD])       # Wk_h^T blocked
    wv_d = din("wv", [128, NB, HD])       # Wv_h^T blocked
    wo_d = din("wo", [128, NB, D])        # Wo^T blocked
    bqk_d = din("bqk", [128, 1], dt.float32)  # [s*bq_h ; bk_h]
    bv_d = din("bv", [1, HD])
    bo_d = din("bo", [1, D])
    ones_d = din("ones", [1, 128])
    eye_d = din("eye", [128, 128], dt.float32)
    sgT_d = din("sgT", [N, RS])           # sgconv_mat[rows,:].T per core
    out_d = nc.dram_tensor("out", [RS, D], dt.float32, kind="ExternalOutput").ap()

    with tile.TileContext(nc) as tc:
        with tc.tile_pool(name="const", bufs=1) as cp, \
             tc.tile_pool(name="persist", bufs=1) as pp, \
             tc.tile_pool(name="dram", bufs=1, space="DRAM") as dp:
            wq_sb = cp.tile([128, NB, HD], dt.bfloat16)
            wk_sb = cp.tile([128, NB, HD], dt.bfloat16)
            wv_sb = cp.tile([128, NB, HD], dt.bfloat16)
            wo_sb = cp.tile([128, NB, D], dt.bfloat16)
            bqk_sb = cp.tile([128, 1], dt.float32)
            bv_sb = cp.tile([1, HD], dt.bfloat16)
            bo_sb = cp.tile([1, D], dt.bfloat16)
            ones_sb = cp.tile([1, 128], dt.bfloat16)
            eye_sb = cp.tile([128, 128], dt.float32)
            for sb_t, d_t in [(wq_sb, wq_d), (wk_sb, wk_d), (wv_sb, wv_d),
                              (wo_sb, wo_d), (bqk_sb, bqk_d), (bv_sb, bv_d),
                              (bo_sb, bo_d), (ones_sb, ones_d), (eye_sb, eye_d)]:
                nc.sync.dma_start(sb_t[:], d_t[:])

            # persistent SBUF state
            T1 = pp.tile([128, N], dt.bfloat16)    # [qT_lo ; kT_hi]
            T2 = pp.tile([128, N], dt.bfloat16)    # [kT_lo ; qT_hi] (swap of T1)
            vh = pp.tile([128, KT, HD + 1], dt.bfloat16)  # [key, kt, hd|ones]
            attn_sb = pp.tile([128, KT, HD], dt.bfloat16)  # [q%128, qt, hd]
            sgb = pp.tile([128, KT, RS], dt.bfloat16)      # [j%128, jt, i]

            nc.vector.memset(vh[:, :, HD:HD + 1], 1.0)

            bounce_a = dp.tile([N // 2, HD], dt.bfloat16)
            bounce_b = dp.tile([N // 2, HD], dt.bfloat16)
            gath_a = dp.tile([NC * (N // 2), HD], dt.bfloat16, addr_space="Shared")
            gath_b = dp.tile([NC * (N // 2), HD], dt.bfloat16, addr_space="Shared")

            # ---------------- Phase A: q/k projections (col-tiled pair) ----
            with tc.tile_pool(name="pa_in", bufs=3) as pa_in, \
                 tc.tile_pool(name="pa_ps", bufs=2, space="PSUM") as pa_ps:
                for nb in range(QB):
                    sl = slice(nb * 512, (nb + 1) * 512)
                    qTb = pa_in.tile([128, NB, 512], dt.bfloat16, tag="q")
                    kTb = pa_in.tile([128, NB, 512], dt.bfloat16, tag="k")
                    nc.sync.dma_start(qTb[:], qT_d[:, :, sl])
                    nc.sync.dma_start(kTb[:], kT_d[:, :, sl])
                    ps = pa_ps.tile([128, 512], dt.float32, tag="pa")
                    for kb in range(NB):
                        nc.tensor.matmul(ps[0:64, :], wq_sb[:, kb, :],
                                         qTb[:, kb, :], start=(kb == 0),
                                         stop=(kb == NB - 1),
                                         tile_position=(0, 0))
                        nc.tensor.matmul(ps[64:128, :], wk_sb[:, kb, :],
                                         kTb[:, kb, :], start=(kb == 0),
                                         stop=(kb == NB - 1),
                                         tile_position=(0, 64),
                                         skip_group_check=True)
                    nc.scalar.activation(T1[:, sl], ps[:], Ident, bias=bqk_sb[:])
                    # partition-swapped copy for the row-packed S^T tiles
                    nc.sync.dma_start(T2[0:64, sl], T1[64:128, sl])
                    nc.sync.dma_start(T2[64:128, sl], T1[0:64, sl])

                # ------------- Phase Av: v projection (direct [n, hd]) -----
                with tc.tile_pool(name="pv_in", bufs=3) as pv_in, \
                     tc.tile_pool(name="pv_ps", bufs=3, space="PSUM") as pv_ps:
                    for nb in range(QB):
                        sl = slice(nb * 512, (nb + 1) * 512)
                        vTb = pv_in.tile([128, NB, 512], dt.bfloat16, tag="v")
                        nc.sync.dma_start(vTb[:], vT_d[:, :, sl])
                        for t in range(4):
                            nt = nb * 4 + t
                            psv = pv_ps.tile([128, HD], dt.float32, tag="pv")
                            for kb in range(NB):
                                nc.tensor.matmul(
                                    psv[:], vTb[:, kb, t * 128:(t + 1) * 128],
                                    wv_sb[:, kb, :], start=(kb == 0), stop=False)
                            nc.tensor.matmul(psv[:], ones_sb[0:1, :], bv_sb[:],
                                             start=False, stop=True)
                            nc.vector.tensor_copy(vh[:, nt, 0:HD], psv[:])

            # sgconv matrix load (emitted late so input DMAs win early bandwidth)
            nc.sync.dma_start(sgb[:], sgT_d[:].rearrange("(jt p) i -> p jt i", jt=KT))

            # ---------------- Phase B: attention ----------------
            with tc.tile_pool(name="s_ps", bufs=3, space="PSUM") as s_ps_pool, \
                 tc.tile_pool(name="o_ps", bufs=1, space="PSUM") as o_ps_pool, \
                 tc.tile_pool(name="t_ps", bufs=1, space="PSUM") as t_ps_pool, \
                 tc.tile_pool(name="pt", bufs=3) as pt_pool, \
                 tc.tile_pool(name="ot", bufs=2) as ot_pool, \
                 tc.tile_pool(name="sc", bufs=4) as sc_pool:
                for qb in range(QB):
                    qsl = slice(qb * 512, (qb + 1) * 512)
                    o_ps = o_ps_pool.tile([HD + 1, 512], dt.float32, tag="ops")
                    for g in range(NPAIR):
                        ktA, ktB = 2 * g, 2 * g + 1
                        sps = s_ps_pool.tile([128, 1024], dt.float32, tag="sps")
                        nc.tensor.matmul(
                            sps[:, 0:512], T2[0:64, ktA * 128:(ktA + 1) * 128],
                            T1[0:64, qsl], start=True, stop=True,
                            tile_position=(0, 0))
                        nc.tensor.matmul(
                            sps[:, 512:1024], T1[64:128, ktB * 128:(ktB + 1) * 128],
                            T2[64:128, qsl], start=True, stop=True,
                            tile_position=(64, 0), skip_group_check=True)
                        p = pt_pool.tile([128, 1024], dt.bfloat16, tag="pt")
                        if g < ACT_PAIRS:
                            nc.scalar.activation(p[:], sps[:], Exp)
                        else:
                            nc.vector.tensor_scalar(
                                p[:].bitcast(dt.int16), sps[:],
                                FEXP_A, FEXP_B, Mult, Add)
                        nc.tensor.matmul(o_ps[:], vh[:, ktA, :], p[:, 0:512],
                                         start=(g == 0), stop=False,
                                         skip_group_check=True)
                        nc.tensor.matmul(o_ps[:], vh[:, ktB, :], p[:, 512:1024],
                                         start=False, stop=(g == NPAIR - 1),
                                         skip_group_check=True)
                    ot = ot_pool.tile([HD + 1, 512], dt.float32, tag="ot")
                    nc.vector.tensor_copy(ot[:], o_ps[:])
                    for qs in range(4):
                        t_ps = t_ps_pool.tile([128, HD + 1], dt.float32, tag="tps")
                        nc.tensor.transpose(t_ps[:], ot[:, qs * 128:(qs + 1) * 128],
                                            eye_sb[0:HD + 1, 0:HD + 1])
                        rc = sc_pool.tile([128, 1], dt.float32, tag="rc")
                        nc.vector.reciprocal(rc[:], t_ps[:, HD:HD + 1])
                        nc.vector.tensor_scalar_mul(
                            attn_sb[:, qb * 4 + qs, :], t_ps[:, 0:HD], rc[:])
                    if qb == 3:
                        nc.sync.dma_start(
                            bounce_a[:].rearrange("(qt p) hd -> p qt hd", qt=16),
                            attn_sb[:, 0:16, :])
                        nc.gpsimd.collective_compute(
                            "AllGather", mybir.AluOpType.bypass,
                            replica_groups=[list(range(NC))],
                            ins=[bounce_a[:].opt()], outs=[gath_a[:].opt()])
                    if qb == 7:
                        nc.sync.dma_start(
                            bounce_b[:].rearrange("(qt p) hd -> p qt hd", qt=16),
                            attn_sb[:, 16:32, :])
                        nc.gpsimd.collective_compute(
                            "AllGather", mybir.AluOpType.bypass,
                            replica_groups=[list(range(NC))],
                            ins=[bounce_b[:].opt()], outs=[gath_b[:].opt()])

            # ---------------- Phase C: sgconv (out_sg^T) ----------------
            # gathered attn: [8 heads][2048 rows][64] per half; lhsT tile for
            # row-tile jt needs [128 j, 128 d] with d = (h, hd) interleaved.
            with tc.tile_pool(name="aj", bufs=3) as aj_pool, \
                 tc.tile_pool(name="og_ps", bufs=1, space="PSUM") as og_pool, \
                 tc.tile_pool(name="pd_sb", bufs=1) as pd_sb_pool:
                og = og_pool.tile([128, NB, RS], dt.float32)
                for half, gath in ((0, gath_a), (1, gath_b)):
                    gr = gath[:].rearrange("(h jt p) hd -> p jt h hd", h=NC, jt=16)
                    for j in range(16):
                        jt = half * 16 + j
                        aj = aj_pool.tile([128, NC, HD], dt.bfloat16, tag="aj")
                        nc.sync.dma_start(aj[:], gr[:, j, :, :])
                        for db in range(NB):
                            nc.tensor.matmul(
                                og[:, db, :], aj[:, 2 * db:2 * db + 2, :],
                                sgb[:, jt, :], start=(jt == 0), stop=(jt == KT - 1),
                                skip_group_check=True)
                # ---------------- Phase D: final projection ----------------
                ogT = pd_sb_pool.tile([128, NB, RS], dt.bfloat16)
                for db in range(NB):
                    nc.vector.tensor_copy(ogT[:, db, :], og[:, db, :])
                with tc.tile_pool(name="pd_ps", bufs=2, space="PSUM") as pd_ps_pool, \
                     tc.tile_pool(name="po_sb", bufs=2) as po_sb_pool:
                    for it in range(NB):
                        ps = pd_ps_pool.tile([128, D], dt.float32, tag="pd")
                        for db in range(NB):
                            nc.tensor.matmul(
                                ps[:], ogT[:, db, it * 128:(it + 1) * 128],
                                wo_sb[:, db, :], start=(db == 0), stop=False)
                        nc.tensor.matmul(ps[:], ones_sb[:], bo_sb[:],
                                         start=False, stop=True)
                        po = po_sb_pool.tile([128, D], dt.float32, tag="po")
                        nc.vector.tensor_copy(po[:], ps[:])
                        nc.sync.dma_start(out_d[it * 128:(it + 1) * 128, :], po[:])
    nc.compile()
    return nc


def kernel(**inputs):
    query = np.asarray(inputs["query"], dtype=np.float32)
    key = np.asarray(inputs["key"], dtype=np.float32)
    value = np.asarray(inputs["value"], dtype=np.float32)
    Wq, bq = np.asarray(inputs["Wq"], np.float32), np.asarray(inputs["bq"], np.float32)
    Wk, bk = np.asarray(inputs["Wk"], np.float32), np.asarray(inputs["bk"], np.float32)
    Wv, bv = np.asarray(inputs["Wv"], np.float32), np.asarray(inputs["bv"], np.float32)
    Wo, bo = np.asarray(inputs["Wo"], np.float32), np.asarray(inputs["bo"], np.float32)
    sg = np.asarray(inputs["sgconv_mat"], np.float32)[0]   # [N, N]

    if "nc" not in _CACHE:
        _CACHE["nc"] = _build()
    nc = _CACHE["nc"]

    qTb = blk(query[0].T.astype(bf16))
    kTb = blk(key[0].T.astype(bf16))
    vTb = blk(value[0].T.astype(bf16))
    wob = blk(Wo.T.astype(bf16))
    common = {
        "qT": qTb, "kT": kTb, "vT": vTb, "wo": wob,
        "bo": bo.reshape(1, D).astype(bf16),
        "ones": np.ones((1, 128), bf16),
        "eye": np.eye(128, dtype=np.float32),
    }
    in_maps = []
    for c in range(NC):
        hs = slice(c * HD, (c + 1) * HD)
        rs = slice(c * RS, (c + 1) * RS)
        in_maps.append(dict(
            common,
            wq=blk((SCALE * Wq[hs, :]).T.astype(bf16)),
            wk=blk(Wk[hs, :].T.astype(bf16)),
            wv=blk(Wv[hs, :].T.astype(bf16)),
            bqk=np.concatenate([SCALE * bq[hs], bk[hs]]).reshape(128, 1)
                .astype(np.float32),
            bv=bv[hs].reshape(1, HD).astype(bf16),
            sgT=np.ascontiguousarray(sg[rs, :].T).astype(bf16),
        ))
    res = run_bass_kernel_spmd(nc, in_maps, core_ids=list(range(NC)),
                               **_CACHE.get("run_kwargs", {}))
    _CACHE["last_results"] = res
    out = np.concatenate([res.results[c]["out"] for c in range(NC)], axis=0)
    return out.reshape(1, N, D)


# revision 4
# speedup vs baseline: 1.9691x; 1.0770x over previous
"""GraphWeightedMHA on 8 trn2 cores — head-sharded bf16 Bass/Tile kernel.

Sharding: one attention head per core (tensor parallel). Each core projects
q/k/v for ALL 4096 sequence positions but only its head's 64 dims — no k/v
collective needed. Attention: S^T tiles via 2-way row-packed PE matmuls
(concurrent 64-row tiles), exp split between the scalar engine (exact) and a
DVE Schraudolph fast-exp (bf16 bit trick), PV with ones-augmented V for the
softmax denominators. The PE instruction stream is software-pipelined: S^T
pairs run 3 ahead of the PV consumers so exp latency never blocks the PE
queue, and the previous block's transpose/normalize is slotted into the next
block's stream. The per-head [4096, 64] attention output is AllGathered in
two halves (overlapped); sgconv + final projection are row-sharded.

All matmuls bf16 (fp32 PSUM accumulation) — fp32r would compile to the ~3x
slower fp32_mode=HIGH multi-pass path.
"""
import numpy as np
import ml_dtypes
import concourse.bass as bass
import concourse.bacc as bacc
import concourse.tile as tile
from concourse import mybir
from concourse.bass_utils import run_bass_kernel_spmd

dt = mybir.dt
bf16 = ml_dtypes.bfloat16
NC = 8
N, D, H, HD = 4096, 512, 8, 64
RS = N // NC          # 512 output rows per core for sgconv/final proj
NB = D // 128         # 4 blocks of 128 along D
QB = 8                # query blocks of 512
KT = N // 128         # 32 key tiles
NPAIR = KT // 2       # 16 row-packed S^T pairs per query block
LOOK = 3              # S^T pair lookahead ahead of PV in the PE stream
SCALE = float(1.0 / np.sqrt(np.float32(D)))
Exp = mybir.ActivationFunctionType.Exp
Ident = mybir.ActivationFunctionType.Identity
Mult = mybir.AluOpType.mult
Add = mybir.AluOpType.add

# Schraudolph fast-exp in bf16 bit space: bf16_bits(exp(x)) ~ round(A*x + B)
FEXP_A = float(np.float32(128.0 / np.log(2.0)))
FEXP_B = float(np.float32(127 * 128 - 5.0))
# which of the 16 pairs per block use the scalar engine's exact exp (rest:
# DVE fast-exp). 9/16 on ACT balances ACT vs DVE busy time.
ACT_SET = {0, 2, 4, 6, 8, 10, 12, 14, 15}

_CACHE: dict = {}


def blk(x):  # [512, M] -> [128, 4, M]  (p, kb, m) with d = kb*128+p
    return np.ascontiguousarray(x.reshape(NB, 128, -1).transpose(1, 0, 2))


def _build():
    nc = bacc.Bacc("TRN2", target_bir_lowering=False, debug=False, num_devices=NC)

    def din(name, shape, d=dt.bfloat16):
        return nc.dram_tensor(name, shape, d, kind="ExternalInput").ap()

    qT_d = din("qT", [128, NB, N])        # query^T blocked (shared)
    kT_d = din("kT", [128, NB, N])        # key^T blocked (shared)
    vT_d = din("vT", [128, NB, N])        # value^T blocked (shared)
    wq_d = din("wq", [128, NB, HD])       # (s*Wq_h)^T blocked
    wk_d = din("wk", [128, NB, HD])       # Wk_h^T blocked
    wv_d = din("wv", [128, NB, HD])       # Wv_h^T blocked
    wo_d = din("wo", [128, NB, D])        # Wo^T blocked
    bqk_d = din("bqk", [128, 1], dt.float32)  # [s*bq_h ; bk_h]
    bv_d = din("bv", [1, HD])
    bo_d = din("bo", [1, D])
    ones_d = din("ones", [1, 128])
    eye_d = din("eye", [128, 128], dt.float32)
    sgT_d = din("sgT", [N, RS])           # sgconv_mat[rows,:].T per core
    out_d = nc.dram_tensor("out", [RS, D], dt.float32, kind="ExternalOutput").ap()

    with tile.TileContext(nc) as tc:
        with tc.tile_pool(name="const", bufs=1) as cp, \
             tc.tile_pool(name="persist", bufs=1) as pp, \
             tc.tile_pool(name="dram", bufs=1, space="DRAM") as dp:
            wq_sb = cp.tile([128, NB, HD], dt.bfloat16)
            wk_sb = cp.tile([128, NB, HD], dt.bfloat16)
            wv_sb = cp.tile([128, NB, HD], dt.bfloat16)
            wo_sb = cp.tile([128, NB, D], dt.bfloat16)
            bqk_sb = cp.tile([128, 1], dt.float32)
            bv_sb = cp.tile([1, HD], dt.bfloat16)
            bo_sb = cp.tile([1, D], dt.bfloat16)
            ones_sb = cp.tile([1, 128], dt.bfloat16)
            eye_sb = cp.tile([128, 128], dt.float32)
            for sb_t, d_t in [(wq_sb, wq_d), (wk_sb, wk_d), (wv_sb, wv_d),
                              (wo_sb, wo_d), (bqk_sb, bqk_d), (bv_sb, bv_d),
                              (bo_sb, bo_d), (ones_sb, ones_d), (eye_sb, eye_d)]:
                nc.sync.dma_start(sb_t[:], d_t[:])

            sgb = pp.tile([128, KT, RS], dt.bfloat16)      # [j%128, jt, i]

            bounce_a = dp.tile([N // 2, HD], dt.bfloat16)
            bounce_b = dp.tile([N // 2, HD], dt.bfloat16)
            gath_a = dp.tile([NC * (N // 2), HD], dt.bfloat16, addr_space="Shared")
            gath_b = dp.tile([NC * (N // 2), HD], dt.bfloat16, addr_space="Shared")

            with tc.tile_pool(name="attn_state", bufs=1) as ap:
                kT_sb = ap.tile([128, NB, N], dt.bfloat16)
                vT_sb = ap.tile([128, NB, N], dt.bfloat16)
                qT_sb = ap.tile([128, NB, N], dt.bfloat16)
                T1 = ap.tile([128, N], dt.bfloat16)    # [qT_lo ; kT_hi]
                T2 = ap.tile([128, N], dt.bfloat16)    # [kT_lo ; qT_hi]
                vh = ap.tile([128, KT, HD + 1], dt.bfloat16)
                attn_sb = ap.tile([128, KT, HD], dt.bfloat16)
                scr = ap.tile([1, 16], dt.float32)

                # input prefetch: k first (S^T needs all keys), then v, then q
                for c in range(4):
                    sl = slice(c * 1024, (c + 1) * 1024)
                    nc.sync.dma_start(kT_sb[:, :, sl], kT_d[:, :, sl])
                for c in range(4):
                    sl = slice(c * 1024, (c + 1) * 1024)
                    nc.sync.dma_start(vT_sb[:, :, sl], vT_d[:, :, sl])
                for c in range(4):
                    sl = slice(c * 1024, (c + 1) * 1024)
                    nc.sync.dma_start(qT_sb[:, :, sl], qT_d[:, :, sl])
                nc.vector.memset(vh[:, :, HD:HD + 1], 1.0)
                # preload the exp table set on ACT while DMAs run
                nc.scalar.activation(scr[:], eye_sb[0:1, 0:16], Exp)

                # ---------------- Phase A: q/k projections (col-tiled) -----
                with tc.tile_pool(name="pa_ps", bufs=2, space="PSUM") as pa_ps:
                    for nb in range(QB):
                        sl = slice(nb * 512, (nb + 1) * 512)
                        ps = pa_ps.tile([128, 512], dt.float32, tag="pa")
                        for kb in range(NB):
                            nc.tensor.matmul(ps[0:64, :], wq_sb[:, kb, :],
                                             qT_sb[:, kb, sl], start=(kb == 0),
                                             stop=(kb == NB - 1),
                                             tile_position=(0, 0))
                            nc.tensor.matmul(ps[64:128, :], wk_sb[:, kb, :],
                                             kT_sb[:, kb, sl], start=(kb == 0),
                                             stop=(kb == NB - 1),
                                             tile_position=(0, 64),
                                             skip_group_check=True)
                        nc.scalar.activation(T1[:, sl], ps[:], Ident,
                                             bias=bqk_sb[:])
                        nc.sync.dma_start(T2[0:64, sl], T1[64:128, sl])
                        nc.sync.dma_start(T2[64:128, sl], T1[0:64, sl])

                    # ------------- Phase Av: v projection (direct [n, hd]) -
                    with tc.tile_pool(name="pv_ps", bufs=3, space="PSUM") as pv_ps:
                        for nt in range(KT):
                            psv = pv_ps.tile([128, HD], dt.float32, tag="pv")
                            for kb in range(NB):
                                nc.tensor.matmul(
                                    psv[:],
                                    vT_sb[:, kb, nt * 128:(nt + 1) * 128],
                                    wv_sb[:, kb, :], start=(kb == 0), stop=False)
                            nc.tensor.matmul(psv[:], ones_sb[0:1, :], bv_sb[:],
                                             start=False, stop=True)
                            nc.vector.tensor_copy(vh[:, nt, 0:HD], psv[:])

                # sgconv matrix load (late: input DMAs get early bandwidth)
                nc.sync.dma_start(
                    sgb[:], sgT_d[:].rearrange("(jt p) i -> p jt i", jt=KT))

                # ---------------- Phase B: attention ----------------
                with tc.tile_pool(name="s_ps", bufs=3, space="PSUM") as s_pool, \
                     tc.tile_pool(name="o_ps", bufs=1, space="PSUM") as o_pool, \
                     tc.tile_pool(name="t_ps", bufs=1, space="PSUM") as t_pool, \
                     tc.tile_pool(name="pt", bufs=3) as pt_pool, \
                     tc.tile_pool(name="ot", bufs=2) as ot_pool, \
                     tc.tile_pool(name="sc", bufs=8) as sc_pool:

                    def emit_S(qsl, g):
                        ktA, ktB = 2 * g, 2 * g + 1
                        sps = s_pool.tile([128, 1024], dt.float32, tag="sps")
                        nc.tensor.matmul(
                            sps[:, 0:512], T2[0:64, ktA * 128:(ktA + 1) * 128],
                            T1[0:64, qsl], start=True, stop=True,
                            tile_position=(0, 0))
                        nc.tensor.matmul(
                            sps[:, 512:1024],
                            T1[64:128, ktB * 128:(ktB + 1) * 128],
                            T2[64:128, qsl], start=True, stop=True,
                            tile_position=(64, 0), skip_group_check=True)
                        return sps

                    norm_steps = []  # deferred prev-block normalize emitters

                    def pop_norm():
                        if norm_steps:
                            norm_steps.pop(0)()

                    def make_norm(qb, ot):
                        def step(qs):
                            def f():
                                t_ps = t_pool.tile([128, HD + 1], dt.float32,
                                                   tag="tps")
                                nc.tensor.transpose(
                                    t_ps[:], ot[:, qs * 128:(qs + 1) * 128],
                                    eye_sb[0:HD + 1, 0:HD + 1])
                                rc = sc_pool.tile([128, 1], dt.float32, tag="rc")
                                nc.vector.reciprocal(rc[:], t_ps[:, HD:HD + 1])
                                nc.vector.tensor_scalar_mul(
                                    attn_sb[:, qb * 4 + qs, :],
                                    t_ps[:, 0:HD], rc[:])
                            return f
                        return [step(qs) for qs in range(4)]

                    for qb in range(QB):
                        qsl = slice(qb * 512, (qb + 1) * 512)
                        o_ps = o_pool.tile([HD + 1, 512], dt.float32, tag="ops")
                        store = {g: emit_S(qsl, g) for g in range(LOOK)}
                        for g in range(NPAIR):
                            sps = store.pop(g)
                            p = pt_pool.tile([128, 1024], dt.bfloat16, tag="pt")
                            if g in ACT_SET:
                                nc.scalar.activation(p[:], sps[:], Exp)
                            else:
                                nc.vector.tensor_scalar(
                                    p[:].bitcast(dt.int16), sps[:],
                                    FEXP_A, FEXP_B, Mult, Add)
                            if g + LOOK < NPAIR:
                                store[g + LOOK] = emit_S(qsl, g + LOOK)
                            pop_norm()  # prev-block transpose rides the stream
                            nc.tensor.matmul(o_ps[:], vh[:, 2 * g, :],
                                             p[:, 0:512], start=(g == 0),
                                             stop=False, skip_group_check=True)
                            nc.tensor.matmul(o_ps[:], vh[:, 2 * g + 1, :],
                                             p[:, 512:1024], start=False,
                                             stop=(g == NPAIR - 1),
                                             skip_group_check=True)
                        ot = ot_pool.tile([HD + 1, 512], dt.float32, tag="ot")
                        nc.scalar.activation(ot[:], o_ps[:], Ident)
                        norm_steps.extend(make_norm(qb, ot))
                        if qb == 3 or qb == 7:
                            while norm_steps:
                                pop_norm()
                            half = (bounce_a, gath_a) if qb == 3 else \
                                   (bounce_b, gath_b)
                            qt0 = 0 if qb == 3 else 16
                            nc.sync.dma_start(
                                half[0][:].rearrange("(qt p) hd -> p qt hd",
                                                     qt=16),
                                attn_sb[:, qt0:qt0 + 16, :])
                            nc.gpsimd.collective_compute(
                                "AllGather", mybir.AluOpType.bypass,
                                replica_groups=[list(range(NC))],
                                ins=[half[0][:].opt()], outs=[half[1][:].opt()])

            # ---------------- Phase C: sgconv (out_sg^T) ----------------
            with tc.tile_pool(name="aj", bufs=2) as aj_pool, \
                 tc.tile_pool(name="og_ps", bufs=1, space="PSUM") as og_pool, \
                 tc.tile_pool(name="pd_sb", bufs=1) as pd_sb_pool:
                og = og_pool.tile([128, NB, RS], dt.float32)
                for half, gath in ((0, gath_a), (1, gath_b)):
                    gr = gath[:].rearrange("(h jt p) hd -> p jt h hd", h=NC, jt=16)
                    aj = aj_pool.tile([128, 16, NC, HD], dt.bfloat16, tag="aj")
                    for j in range(16):  # prefetch: all emitted before the mms
                        nc.sync.dma_start(aj[:, j, :, :], gr[:, j, :, :])
                    for j in range(16):
                        jt = half * 16 + j
                        for db in range(NB):
                            nc.tensor.matmul(
                                og[:, db, :], aj[:, j, 2 * db:2 * db + 2, :],
                                sgb[:, jt, :], start=(jt == 0),
                                stop=(jt == KT - 1), skip_group_check=True)
                # ---------------- Phase D: final projection ----------------
                ogT = pd_sb_pool.tile([128, NB, RS], dt.bfloat16)
                for db in range(NB):
                    nc.vector.tensor_copy(ogT[:, db, :], og[:, db, :])
                with tc.tile_pool(name="pd_ps", bufs=2, space="PSUM") as pd_ps_pool, \
                     tc.tile_pool(name="po_sb", bufs=2) as po_sb_pool:
                    for it in range(NB):
                        ps = pd_ps_pool.tile([128, D], dt.float32, tag="pd")
                        for db in range(NB):
                            nc.tensor.matmul(
                                ps[:], ogT[:, db, it * 128:(it + 1) * 128],
                                wo_sb[:, db, :], start=(db == 0), stop=False)
                        nc.tensor.matmul(ps[:], ones_sb[:], bo_sb[:],
                                         start=False, stop=True)
                        po = po_sb_pool.tile([128, D], dt.float32, tag="po")
                        nc.vector.tensor_copy(po[:], ps[:])
                        nc.sync.dma_start(out_d[it * 128:(it + 1) * 128, :], po[:])
    nc.compile()
    return nc


def kernel(**inputs):
    query = np.asarray(inputs["query"], dtype=np.float32)
    key = np.asarray(inputs["key"], dtype=np.float32)
    value = np.asarray(inputs["value"], dtype=np.float32)
    Wq, bq = np.asarray(inputs["Wq"], np.float32), np.asarray(inputs["bq"], np.float32)
    Wk, bk = np.asarray(inputs["Wk"], np.float32), np.asarray(inputs["bk"], np.float32)
    Wv, bv = np.asarray(inputs["Wv"], np.float32), np.asarray(inputs["bv"], np.float32)
    Wo, bo = np.asarray(inputs["Wo"], np.float32), np.asarray(inputs["bo"], np.float32)
    sg = np.asarray(inputs["sgconv_mat"], np.float32)[0]   # [N, N]

    if "nc" not in _CACHE:
        _CACHE["nc"] = _build()
    nc = _CACHE["nc"]

    qTb = blk(query[0].T.astype(bf16))
    kTb = blk(key[0].T.astype(bf16))
    vTb = blk(value[0].T.astype(bf16))
    wob = blk(Wo.T.astype(bf16))
    common = {
        "qT": qTb, "kT": kTb, "vT": vTb, "wo": wob,
        "bo": bo.reshape(1, D).astype(bf16),
        "ones": np.ones((1, 128), bf16),
        "eye": np.eye(128, dtype=np.float32),
    }
    in_maps = []
    for c in range(NC):
        hs = slice(c * HD, (c + 1) * HD)
        rs = slice(c * RS, (c + 1) * RS)
        in_maps.append(dict(
            common,
            wq=blk((SCALE * Wq[hs, :]).T.astype(bf16)),
            wk=blk(Wk[hs, :].T.astype(bf16)),
            wv=blk(Wv[hs, :].T.astype(bf16)),
            bqk=np.concatenate([SCALE * bq[hs], bk[hs]]).reshape(128, 1)
                .astype(np.float32),
            bv=bv[hs].reshape(1, HD).astype(bf16),
            sgT=np.ascontiguousarray(sg[rs, :].T).astype(bf16),
        ))
    res = run_bass_kernel_spmd(nc, in_maps, core_ids=list(range(NC)),
                               **_CACHE.get("run_kwargs", {}))
    _CACHE["last_results"] = res
    out = np.concatenate([res.results[c]["out"] for c in range(NC)], axis=0)
    return out.reshape(1, N, D)
